# revision 4
# baseline (speedup 1.0000x reference)
"""Trainium2 Bass kernel for the two-branch (spatial/temporal) attention module.

Computation (full, fp32 reference):
    qkv = x @ Wqkv; q,k,v split -> heads [b,8,n,64]; half = n//2
    all 4096 queries attend to k_t (keys 2048:4096); softmax; out rows
    0:2048 read v rows 0:2048 (spatial), rows 2048:4096 read v rows
    2048:4096 (temporal); concat heads; out @ Wout + b_out.

Sharding (8 cores): core c handles batch c//4 and heads {2*(c%4), 2*(c%4)+1}.
Each core computes its 2 heads' q/k/v projections (tensor-parallel columns of
Wqkv), full attention for those heads, and a partial output projection using
its 128 rows of Wout. The host sums the 4 partial outputs per batch (the
"all-reduce") and adds b_out.

On-chip layout is fully transposed (feature dims on partitions) so no big
transposes are needed:
  xT [512,4096] -> qT/kT/vT [128(2h x 64d), 4096n] via lhsT=Wqkv tiles
  dots_T [j,i] via lhsT=kT-tile, rhs=qT         (K=64 contraction)
  E = exp(SCALE*dots_T) on ScalarE (no max subtraction needed: |logits|<~3)
  out/sums via lhsT=[v|1] (v natural from small PE transposes), rhs=E
  normalization via DVE reciprocal + GpSimd partition broadcast
  P^T [e,n] via lhsT=Wout-tile, rhs=A^T
"""

import sys

sys.path.insert(0, "/opt/trn_rl_repo")

import numpy as np

import concourse.bass as bass
import concourse.mybir as mybir
import concourse.tile as tile
from concourse import bacc
from concourse.bass_utils import run_bass_kernel_spmd
from concourse.masks import make_identity

F32 = mybir.dt.float32
F32R = mybir.dt.float32r
BF16 = mybir.dt.bfloat16

N = 4096
HALF = 2048
DIM = 512
D = 64  # dim head
SCALE = DIM ** -0.5

# attention compute dtype: BF16 (fast) or F32 (accurate, 4x PE cost via f32r)
ATT_DT = BF16


def _mm_dt(ap):
    """View an fp32 AP as float32r for full-rate fp32 matmuls; bf16 passes through."""
    if ap.dtype == F32:
        return ap.bitcast(F32R)
    return ap


def build_nc():
    nc = bacc.Bacc("TRN2", target_bir_lowering=False, debug=False)

    xT_d = nc.dram_tensor("xT", [DIM, N], F32R, kind="ExternalInput")
    wqkv_d = nc.dram_tensor("Wqkv", [DIM, 384], F32R, kind="ExternalInput")
    wout_d = nc.dram_tensor("Wout", [128, DIM], F32R, kind="ExternalInput")
    outT_d = nc.dram_tensor("outT", [DIM, N], F32, kind="ExternalOutput")

    AF = mybir.ActivationFunctionType

    with tile.TileContext(nc) as tc:
        with tc.tile_pool(name="persist", bufs=1) as persist:
            # q/k/v transposed: rows 0:64 head A, 64:128 head B; cols = n
            qT = persist.tile([128, N], ATT_DT, tag="qT")
            kT = persist.tile([128, N], ATT_DT, tag="kT")
            vT = persist.tile([128, N], ATT_DT, tag="vT")
            # v natural + ones column: [j-part, jtile, head, 65]
            vp = persist.tile([128, 32, 2, 65], ATT_DT, tag="vp")
            wq_s = persist.tile([128, 4, 384], F32R, tag="wq")
            wout_s = persist.tile([128, DIM], F32R, tag="wout")
            ident = persist.tile([128, 128], ATT_DT, tag="ident")
            AT = persist.tile([128, N], F32R, tag="AT")

            # ---------------- load + qkv projection + v transpose ------------
            with (
                tc.tile_pool(name="xpool", bufs=1) as xpool,
                tc.tile_pool(name="pp", bufs=4, space="PSUM") as pp,
                tc.tile_pool(name="pt", bufs=2, space="PSUM") as pt,
            ):
                xt = xpool.tile([128, 4, N], F32R, tag="xt")
                for ct in range(4):
                    nc.sync.dma_start(
                        out=xt[:, ct, :], in_=xT_d[128 * ct : 128 * (ct + 1), :]
                    )
                nc.sync.dma_start(
                    out=wq_s[:, :, :],
                    in_=wqkv_d[:, :].rearrange("(t p) c -> p t c", p=128),
                )
                nc.sync.dma_start(out=wout_s[:, :], in_=wout_d[:, :])
                make_identity(nc, ident[:, :])
                nc.vector.memset(vp[:, :, :, 64:65], 1.0)

                # qkv^T = Wqkv_c^T @ x^T : out[col, n] per 128-col block
                targets = [qT, kT, vT]
                for m in range(3):
                    for cc in range(8):
                        ps = pp.tile([128, 512], F32, tag="pp", name="ps")
                        for ct in range(4):
                            nc.tensor.matmul(
                                out=ps[:, :],
                                lhsT=wq_s[:, ct, 128 * m : 128 * (m + 1)],
                                rhs=xt[:, ct, 512 * cc : 512 * (cc + 1)],
                                start=(ct == 0),
                                stop=(ct == 3),
                            )
                        nc.vector.tensor_copy(
                            out=targets[m][:, 512 * cc : 512 * (cc + 1)], in_=ps[:, :]
                        )

                # v natural layout: transpose vT 128-col blocks; split heads
                for jt in range(32):
                    tp = pt.tile([128, 128], ATT_DT, tag="pt", name="tp")
                    nc.tensor.transpose(
                        tp[:, :], vT[:, 128 * jt : 128 * (jt + 1)], ident[:, :]
                    )
                    nc.vector.tensor_copy(out=vp[:, jt, 0, 0:64], in_=tp[:, 0:64])
                    nc.vector.tensor_copy(out=vp[:, jt, 1, 0:64], in_=tp[:, 64:128])

            # ---------------- attention ------------------------------------
            with (
                tc.tile_pool(name="pd", bufs=2, space="PSUM") as pd,
                tc.tile_pool(name="pa", bufs=2, space="PSUM") as pa,
                tc.tile_pool(name="es", bufs=3) as es,
                tc.tile_pool(name="sm", bufs=2) as sm,
            ):
                for h in range(2):
                    hp = 64 * h
                    for cc in range(4):  # 1024-wide query chunks
                        i0 = 1024 * cc
                        voff = 0 if cc < 2 else 16  # spatial reads v[0:2048], temporal v[2048:]
                        av = pa.tile([128, 1024], F32, tag="pa", name="av")
                        for jt in range(16):
                            dp = pd.tile([128, 1024], F32, tag="pd", name="dp")
                            for hf in range(2):
                                nc.tensor.matmul(
                                    out=dp[:, 512 * hf : 512 * (hf + 1)],
                                    lhsT=_mm_dt(
                                        kT[
                                            hp : hp + 64,
                                            HALF + 128 * jt : HALF + 128 * (jt + 1),
                                        ]
                                    ),
                                    rhs=_mm_dt(
                                        qT[
                                            hp : hp + 64,
                                            i0 + 512 * hf : i0 + 512 * (hf + 1),
                                        ]
                                    ),
                                    start=True,
                                    stop=True,
                                )
                            et = es.tile([128, 1024], ATT_DT, tag="es", name="et")
                            nc.scalar.activation(
                                out=et[:, :], in_=dp[:, :], func=AF.Exp, scale=SCALE
                            )
                            for hf in range(2):
                                nc.tensor.matmul(
                                    out=av[0:65, 512 * hf : 512 * (hf + 1)],
                                    lhsT=_mm_dt(vp[:, voff + jt, h, :]),
                                    rhs=_mm_dt(et[:, 512 * hf : 512 * (hf + 1)]),
                                    start=(jt == 0),
                                    stop=(jt == 15),
                                )
                        # normalize: A^T = av[0:64] / av[64]
                        rr = sm.tile([1, 1024], F32, tag="rr", name="rr")
                        nc.vector.reciprocal(out=rr[:, :], in_=av[64:65, :])
                        rb = sm.tile([64, 1024], F32, tag="rb", name="rb")
                        nc.gpsimd.partition_broadcast(rb[:, :], rr[:, :], channels=64)
                        nc.vector.tensor_mul(
                            out=AT[hp : hp + 64, i0 : i0 + 1024],
                            in0=av[0:64, :],
                            in1=rb[:, :],
                        )

            # ---------------- output projection (partial) -------------------
            with (
                tc.tile_pool(name="po", bufs=4, space="PSUM") as po,
                tc.tile_pool(name="os", bufs=4) as osb,
            ):
                for cc in range(8):
                    for et_ in range(4):
                        ps2 = po.tile([128, 512], F32, tag="po", name="ps2")
                        nc.tensor.matmul(
                            out=ps2[:, :],
                            lhsT=wout_s[:, 128 * et_ : 128 * (et_ + 1)],
                            rhs=AT[:, 512 * cc : 512 * (cc + 1)],
                            start=True,
                            stop=True,
                        )
                        ot = osb.tile([128, 512], F32, tag="os", name="ot")
                        nc.vector.tensor_copy(out=ot[:, :], in_=ps2[:, :])
                        nc.sync.dma_start(
                            out=outT_d[
                                128 * et_ : 128 * (et_ + 1), 512 * cc : 512 * (cc + 1)
                            ],
                            in_=ot[:, :],
                        )

    nc.compile()
    return nc


_NC = None


def _get_nc():
    global _NC
    if _NC is None:
        _NC = build_nc()
    return _NC


def shard_inputs(x, Wqkv, Wout):
    ins = []
    for core in range(8):
        b, cp = core // 4, core % 4
        hA = 2 * cp
        xT = np.ascontiguousarray(np.asarray(x[b], np.float32).T)
        wq = Wqkv[:, 64 * hA : 64 * hA + 128]
        wk = Wqkv[:, 512 + 64 * hA : 512 + 64 * hA + 128]
        wv = Wqkv[:, 1024 + 64 * hA : 1024 + 64 * hA + 128]
        wqkv_c = np.ascontiguousarray(
            np.concatenate([wq, wk, wv], axis=1), dtype=np.float32
        )
        wout_c = np.ascontiguousarray(Wout[128 * cp : 128 * cp + 128, :])
        ins.append({"xT": xT, "Wqkv": wqkv_c, "Wout": wout_c})
    return ins


def run(x, Wqkv, Wout, b_out, trace=False):
    x = np.asarray(x, np.float32)
    Wqkv = np.asarray(Wqkv, np.float32)
    Wout = np.asarray(Wout, np.float32)
    b_out = np.asarray(b_out, np.float32)

    nc = _get_nc()
    ins = shard_inputs(x, Wqkv, Wout)
    res = run_bass_kernel_spmd(nc, ins, list(range(8)), trace=trace)

    out = np.zeros((2, N, DIM), np.float32)
    for core in range(8):
        b = core // 4
        out[b] += res.results[core]["outT"].T
    out += b_out
    return out, res


def kernel(x, Wqkv, Wout, b_out):
    out, _ = run(x, Wqkv, Wout, b_out, trace=False)
    return out
